# revision 6
# baseline (speedup 1.0000x reference)
"""Trainium2 Bass kernel for quantized Linear + ReLU/identity concat.

Computes: lin = dequant(inp) @ dequant(weight).T + bias ; out = [relu(lin), lin]
with per-tensor input quant params and per-output-channel weight quant params.

Strategy (v2)
-------------
Host side (free — not on the HW critical path):
  * zero-point-shift the input ints and cast to bf16 (integers |v| <= 138 are
    exact in bf16, so the x operand is EXACT).
  * fold BOTH scales into the weight: w_fold[n,k] = (w[n,k]-zw[n])*sw[n]*si,
    rounded once to bf16 (~0.15%% absmax rel err, gate is 2e-2). This removes
    the per-tile scale multiply from the device epilogue entirely.
  * swizzle x and w into [partition][chunk][col] blobs so every load DMA has
    >=2KB contiguous runs per partition (descriptor-efficient), and geometric
    granule sizes so the first chunk lands early.
  * outputs are written bf16 in SBUF-native layout; host reassembles + upcasts.

Device side (8 NeuronCores, data-parallel over M rows, no collectives):
  * bf16 matmul, fp32 PSUM accumulation. Per core: 8 m-tiles x 16 k-chunks x
    4 n-blocks of [128,512] = 512 matmuls (the PE roofline, ~109us warm).
  * Phase A (n-block 0): k-outer over all 8 m-tiles -> only 384KB of input
    needed per 1.7us of PE work, so the PE never waits on the weight stream.
  * Phases B1..B3 (n-blocks 1..3): per m-tile k-contiguous accumulation,
    staggered bank completions, weights fully resident by then.
  * epilogue per (m,nb): one DVE add (psum + bias -> bf16 stage), one ACT
    relu; stores batched per 4 m-tiles (half-phase) on the DVE/ACT rings.
"""

import os
from contextlib import ExitStack

import ml_dtypes
import numpy as np

import concourse.bass as bass  # noqa: F401  (bass types reachable via bacc)
import concourse.mybir as mybir
import concourse.tile as tile
from concourse import bacc
from concourse.bass_utils import run_bass_kernel_spmd

M, K, N = 8192, 2048, 2048
NCORES = 8
MS = M // NCORES  # rows per core
P = 128
NBLK = 512  # matmul moving-operand free dim = one fp32 PSUM bank
KC = K // P  # k chunks of 128
MT = MS // P  # m tiles of 128 per core
NT = N // NBLK  # n blocks of 512

BF16 = ml_dtypes.bfloat16

# uniform 2-chunk load granules: big enough to amortize per-DMA ring
# overheads (~1.5us each), fine enough that phase A never waits
GRANULES = [(k, k + 2) for k in range(0, KC, 2)]

_CACHE: dict = {}
LAST_RESULTS = None  # BassKernelResults of the most recent run (for test.py)


def _build():
    nc = bacc.Bacc("TRN2", target_bir_lowering=False, debug=False, num_devices=NCORES)
    # swizzled inputs: xg[p, kc*MS + m] = x_sh[m, kc*128+p]
    xg = nc.dram_tensor("xg", [P, KC * MS], mybir.dt.bfloat16, kind="ExternalInput")
    # wg[p, ((nb*KC)+kc)*NBLK + n] = w_foldT[kc*128+p, nb*512+n]
    wg = nc.dram_tensor("wg", [P, NT * KC * NBLK], mybir.dt.bfloat16, kind="ExternalInput")
    biasd = nc.dram_tensor("bias", [1, N], mybir.dt.float32, kind="ExternalInput")
    # out chunks: idx = nb*4 + half*2 + branch (0=relu, 1=lin), each [128, 4*512]
    out = nc.dram_tensor("out", [4 * NT, P, 4 * NBLK], mybir.dt.bfloat16,
                         kind="ExternalOutput")
    out_ap = out[:]

    with tile.TileContext(nc) as tc, ExitStack() as ctx:
        const_pool = ctx.enter_context(tc.tile_pool(name="const", bufs=1))
        data_pool = ctx.enter_context(tc.tile_pool(name="data", bufs=1))
        psum_pool = ctx.enter_context(tc.tile_pool(name="psum", bufs=8, space="PSUM"))
        stage_pool = ctx.enter_context(tc.tile_pool(name="stage", bufs=1))

        # --- resident input/weight, loaded in consumption order.
        # Phase-A granules (w-nb0 + x, interleaved) on the SP ring at
        # ~226GB/s demand; bias + the nb1..3 weight blocks on the ACT ring
        # (idle until the first epilogue at ~36us).
        x_all = data_pool.tile([P, KC * MS], mybir.dt.bfloat16, tag="x_all")
        w_all = data_pool.tile([P, NT * KC * NBLK], mybir.dt.bfloat16, tag="w_all")

        bias_row = const_pool.tile([1, N], mybir.dt.float32, tag="bias_row")
        nc.scalar.dma_start(bias_row[:], biasd[:])

        for k0, k1 in GRANULES:
            nc.sync.dma_start(
                w_all[:, k0 * NBLK : k1 * NBLK], wg[:, k0 * NBLK : k1 * NBLK]
            )
            nc.sync.dma_start(
                x_all[:, k0 * MS : k1 * MS], xg[:, k0 * MS : k1 * MS]
            )
        for nb in range(1, NT):
            o = nb * KC * NBLK
            nc.scalar.dma_start(
                w_all[:, o : o + KC * NBLK], wg[:, o : o + KC * NBLK]
            )

        bias_rep = const_pool.tile([P, N], mybir.dt.float32, tag="bias")
        nc.gpsimd.partition_broadcast(bias_rep[:], bias_row[:])

        def lhsT(kci, mi):
            o = kci * MS + mi * P
            return x_all[:, o : o + P]

        def wslice(nb, kci):
            o = (nb * KC + kci) * NBLK
            return w_all[:, o : o + NBLK]

        stage = {}  # (nb, half) -> (lin_tile, rel_tile)

        def epilogue(nb, mi, ps, store_every):
            half, mt = mi // 4, mi % 4
            if mt == 0:
                lin_t = stage_pool.tile([P, 4 * NBLK], mybir.dt.bfloat16,
                                        tag="lin", bufs=3, name=f"lin_{nb}_{half}")
                rel_t = stage_pool.tile([P, 4 * NBLK], mybir.dt.bfloat16,
                                        tag="rel", bufs=3, name=f"rel_{nb}_{half}")
                stage[(nb, half)] = (lin_t, rel_t)
            lin_t, rel_t = stage[(nb, half)]
            ms = slice(mt * NBLK, (mt + 1) * NBLK)
            ns = slice(nb * NBLK, (nb + 1) * NBLK)
            nc.vector.tensor_add(lin_t[:, ms], ps[:], bias_rep[:, ns])
            nc.scalar.activation(rel_t[:, ms], lin_t[:, ms],
                                 mybir.ActivationFunctionType.Relu)
            if (mt + 1) % store_every == 0:
                cs = slice((mt + 1 - store_every) * NBLK, (mt + 1) * NBLK)
                idx_r = nb * 4 + half * 2
                idx_l = idx_r + 1
                nc.scalar.dma_start(out_ap[idx_r, :, cs], rel_t[:, cs])
                nc.sync.dma_start(out_ap[idx_l, :, cs], lin_t[:, cs])

        # --- Phase A: n-block 0, k-outer over all 8 m-tiles (8 PSUM banks).
        psA = [
            psum_pool.tile([P, NBLK], mybir.dt.float32, tag="ps", name=f"psA_{mi}")
            for mi in range(MT)
        ]
        for kci in range(KC):
            for mi in range(MT):
                nc.tensor.matmul(
                    psA[mi][:], lhsT(kci, mi), wslice(0, kci),
                    start=(kci == 0), stop=(kci == KC - 1),
                )
        for mi in range(MT):
            epilogue(0, mi, psA[mi], store_every=4)

        # --- Phases B1..B3: per m-tile k-contiguous groups, staggered banks.
        for nb in range(1, NT):
            store_every = 2 if nb == NT - 1 else 4
            for mi in range(MT):
                ps = psum_pool.tile([P, NBLK], mybir.dt.float32, tag="ps",
                                    name=f"ps_{nb}_{mi}")
                for kci in range(KC):
                    nc.tensor.matmul(
                        ps[:], lhsT(kci, mi), wslice(nb, kci),
                        start=(kci == 0), stop=(kci == KC - 1),
                    )
                epilogue(nb, mi, ps, store_every=store_every)

    nc.compile()
    return nc


def kernel(inp, weight, bias, inp_scales, inp_zero_points, weight_scales, weight_zero_points):
    global LAST_RESULTS
    inp = np.asarray(inp)
    weight = np.asarray(weight)
    bias = np.asarray(bias, dtype=np.float32)
    inp_scales = np.asarray(inp_scales, dtype=np.float32)
    inp_zero_points = np.asarray(inp_zero_points)
    weight_scales = np.asarray(weight_scales, dtype=np.float32)
    weight_zero_points = np.asarray(weight_zero_points)

    zi = int(inp_zero_points.reshape(-1)[0])
    si = float(inp_scales.reshape(-1)[0])
    # fold both scales into the weight; one bf16 rounding (~1.5e-3 rel err)
    w_fold = (
        (weight.astype(np.float64) - weight_zero_points.reshape(-1, 1))
        * (weight_scales.reshape(-1, 1).astype(np.float64) * si)
    ).astype(BF16)
    wT = w_fold.T  # [K, N]
    # wg[p, nb, kc, n] = wT[kc*128+p, nb*512+n]
    wg = np.ascontiguousarray(
        wT.reshape(KC, P, NT, NBLK).transpose(1, 2, 0, 3).reshape(P, NT * KC * NBLK)
    )
    bias2 = bias.reshape(1, N)

    if "nc" not in _CACHE:
        _CACHE["nc"] = _build()
    nc = _CACHE["nc"]

    in_maps = []
    for c in range(NCORES):
        rows = slice(c * MS, (c + 1) * MS)
        x_sh = (inp[rows] - zi).astype(BF16)  # exact small ints
        # xg[p, kc, m] = x_sh[m, kc*128+p]
        xgc = np.ascontiguousarray(
            x_sh.T.reshape(KC, P, MS).transpose(1, 0, 2).reshape(P, KC * MS)
        )
        in_maps.append({"xg": xgc, "wg": wg, "bias": bias2})

    trace = os.environ.get("BASS_TRACE", "0") == "1"
    res = run_bass_kernel_spmd(nc, in_maps, core_ids=list(range(NCORES)), trace=trace)
    LAST_RESULTS = res

    full = np.empty((M, 2 * N), dtype=np.float32)
    for c in range(NCORES):
        arr = np.asarray(res.results[c]["out"])  # [16, 128, 2048] bf16
        for nb in range(NT):
            for half in range(2):
                for br in range(2):  # 0=relu, 1=lin
                    chunk = arr[nb * 4 + half * 2 + br]
                    c4 = (
                        chunk.reshape(P, 4, NBLK)
                        .transpose(1, 0, 2)
                        .reshape(4 * P, NBLK)
                        .astype(np.float32)
                    )
                    r0 = c * MS + half * 4 * P
                    c0 = br * N + nb * NBLK
                    full[r0 : r0 + 4 * P, c0 : c0 + NBLK] = c4
    return full
